# revision 1
# baseline (speedup 1.0000x reference)
"""Causal single-head attention (B=4, T=4096, C=1024, H=128) on 8 Trainium2
NeuronCores.

Sharding: 8 cores = 4 batches x 2 key-parity shards. Each core handles one
batch and the keys in every other 128-block (parity h = core % 2), computing
UN-normalized partial attention (numerator O^T and denominator) for ALL 4096
queries of its batch via unsafe softmax (scores are N(0,1)-bounded, |s| < 8,
so exp never overflows and max-subtraction is unnecessary; partial results
combine exactly by summation across the two cores of a batch).

SPMD uniformity trick: the program is identical on all cores; per-core
differences live entirely in the DATA. The host passes x[b].T with its
columns block-permuted so that this core's keys always sit at the EVEN
128-block positions, plus two per-core causal mask tiles for the two
diagonal blocks of each 512-query group. The host un-permutes the returned
query axis and combines: out = (O0 + O1) / (d0 + d1), transposed.

On-chip layout (everything f32r so matmuls run at full PE rate, N=512):
  S^T blocks [128 keys, 512 queries] = matmul(lhsT=K^T block, rhs=Q^T group)
  E = exp(S^T) on ACT (PSUM->SBUF), masked on DVE for the 2 diagonal blocks
  O^T += matmul(lhsT=V block [k,H], rhs=E)   accumulated in PSUM
  denom = matmul(lhsT=ones[128,1], rhs=sum_i E_i)  (E accumulated on DVE)
"""

import sys
import numpy as np

sys.path.insert(0, "/opt/trn_rl_repo")

B, T, C, H = 4, 4096, 1024, 128
KB = 128            # key block
QG = 512            # query group
NKB = T // KB       # 32 global key blocks
NQG = T // QG       # 8 query groups
NCH = C // 128      # 8 contraction chunks
NST = 4             # supertiles of 1024 positions
SCALE = float(H) ** -0.5

_prog_cache = {}


def _build_program():
    import concourse.mybir as mybir
    import concourse.tile as tile
    from concourse import bacc
    from concourse.masks import make_identity

    F32 = mybir.dt.float32
    F32R = mybir.dt.float32r
    BF16 = mybir.dt.bfloat16
    AF = mybir.ActivationFunctionType

    nc = bacc.Bacc()
    xt = nc.dram_tensor("xt", [C, T], F32R, kind="ExternalInput")
    wq = nc.dram_tensor("wq", [C, H], F32R, kind="ExternalInput")
    wk = nc.dram_tensor("wk", [C, H], F32R, kind="ExternalInput")
    wv = nc.dram_tensor("wv", [C, H], F32R, kind="ExternalInput")
    mp = nc.dram_tensor("mp", [KB, 2 * QG], F32R, kind="ExternalInput")
    ot = nc.dram_tensor("ot", [H, T], F32, kind="ExternalOutput")
    dn = nc.dram_tensor("dn", [1, T], F32, kind="ExternalOutput")

    with tile.TileContext(nc) as tc:
        with (
            tc.tile_pool(name="singles", bufs=1) as singles,
            tc.tile_pool(name="persist", bufs=1) as persist,
            tc.tile_pool(name="xs", bufs=16) as xsp,
            tc.tile_pool(name="epool", bufs=6) as epool,
            tc.tile_pool(name="eacc", bufs=2) as eaccp,
            tc.tile_pool(name="vstage", bufs=2) as vstagep,
            tc.tile_pool(name="outs", bufs=3) as outsp,
            tc.tile_pool(name="pproj", bufs=2, space="PSUM") as pproj,
            tc.tile_pool(name="ps", bufs=4, space="PSUM") as psp,
            tc.tile_pool(name="po", bufs=2, space="PSUM") as pop,
        ):
            # ---- constants ----
            w_sb = {}
            for name, w in (("wq", wq), ("wk", wk), ("wv", wv)):
                t_ = singles.tile([128, NCH, 128], F32R, tag=f"w_{name}")
                nc.sync.dma_start(out=t_, in_=w.rearrange("(c p) h -> p c h", p=128))
                w_sb[name] = t_
            mp_sb = singles.tile([KB, 2 * QG], F32R, tag="mp")
            ones_f = singles.tile([128, 1], F32, tag="ones_f")
            nc.vector.memset(ones_f, 1.0)
            ones_sb = singles.tile([128, 1], F32R, tag="ones")
            nc.scalar.activation(ones_sb, ones_f, AF.Copy)
            ident = singles.tile([128, 128], F32, tag="ident")
            make_identity(nc, ident)

            qT = persist.tile([128, T], F32R, tag="qT")
            kT = persist.tile([128, T // 2], F32R, tag="kT")
            v_sb = persist.tile([128, T // 2], F32R, tag="v")

            def attention_group(j):
                nk = 2 * (j + 1)
                opsum = pop.tile([128, QG], F32, tag="opsum")
                eacc = eaccp.tile([KB, QG], F32R, tag="eacc")
                eacc2 = eaccp.tile([KB, QG], F32R, tag="eacc2")
                qrhs = qT[:, j * QG:(j + 1) * QG]
                es = []

                def emit_pv(i):
                    nc.tensor.matmul(
                        opsum,
                        lhsT=v_sb[:, i * KB:(i + 1) * KB],
                        rhs=es[i],
                        start=(i == 0), stop=(i == nk - 1),
                    )

                for i in range(nk):
                    spsum = psp.tile([KB, QG], F32, tag="spsum")
                    nc.tensor.matmul(
                        spsum,
                        lhsT=kT[:, i * KB:(i + 1) * KB],
                        rhs=qrhs, start=True, stop=True,
                    )
                    # first block's exp lands directly in the accumulator
                    e = eacc if i == 0 else epool.tile([KB, QG], F32R, tag="e")
                    nc.scalar.activation(e, spsum, AF.Exp)
                    if i == nk - 2:
                        nc.vector.tensor_mul(e, e, mp_sb[:, :QG])
                    elif i == nk - 1:
                        nc.vector.tensor_mul(e, e, mp_sb[:, QG:])
                    es.append(e)
                    if i >= 2:
                        emit_pv(i - 2)       # PV lags two stages behind S/exp
                    if i == 1:
                        nc.vector.tensor_copy(eacc2, e)
                    elif i > 1:
                        acc = eacc if i % 2 == 0 else eacc2
                        nc.vector.tensor_add(acc, acc, e)
                emit_pv(nk - 2)
                emit_pv(nk - 1)
                dpsum = pproj.tile([1, QG], F32, tag="proj")
                nc.tensor.matmul(dpsum, lhsT=ones_sb, rhs=eacc,
                                 start=True, stop=False)
                nc.tensor.matmul(dpsum, lhsT=ones_sb, rhs=eacc2,
                                 start=False, stop=True)
                osb = outsp.tile([128, QG], F32, tag="osb")
                nc.vector.tensor_copy(osb, opsum)
                nc.sync.dma_start(out=ot[:, j * QG:(j + 1) * QG], in_=osb)
                dsb = outsp.tile([1, QG], F32, tag="dsb")
                nc.vector.tensor_copy(dsb, dpsum)
                nc.sync.dma_start(out=dn[:, j * QG:(j + 1) * QG], in_=dsb)

            mask_dma_done = []

            for s in range(NST):
                # stream this supertile of x^T: 8 chunk tiles [128, 1024]
                xs = []
                for c in range(NCH):
                    t_ = xsp.tile([128, 1024], F32R, tag="xs")
                    nc.sync.dma_start(
                        out=t_,
                        in_=xt[c * 128:(c + 1) * 128, s * 1024:(s + 1) * 1024],
                    )
                    xs.append(t_)
                if not mask_dma_done:
                    nc.sync.dma_start(out=mp_sb, in_=mp[:])
                    mask_dma_done.append(1)

                def keys_rhs(c):
                    # even 128-blocks of the supertile: cols 0-127, 256-383, ...
                    return xs[c].rearrange(
                        "p (u two b) -> p two u b", two=2, b=128)[:, 0]

                # K^T tile s  [128, 512]
                kpsum = pproj.tile([128, QG], F32, tag="proj")
                for c in range(NCH):
                    nc.tensor.matmul(kpsum, lhsT=w_sb["wk"][:, c], rhs=keys_rhs(c),
                                     start=(c == 0), stop=(c == NCH - 1))
                nc.vector.tensor_copy(kT[:, s * QG:(s + 1) * QG], kpsum)

                # V^T tile s -> transpose into V rows
                vpsum = pproj.tile([128, QG], F32, tag="proj")
                for c in range(NCH):
                    nc.tensor.matmul(vpsum, lhsT=w_sb["wv"][:, c], rhs=keys_rhs(c),
                                     start=(c == 0), stop=(c == NCH - 1))
                vstage = vstagep.tile([128, QG], F32, tag="vstage")
                nc.vector.tensor_copy(vstage, vpsum)
                for u in range(4):
                    tpsum = pproj.tile([128, 128], F32, tag="proj")
                    nc.tensor.transpose(tpsum, vstage[:, u * 128:(u + 1) * 128],
                                        ident)
                    nc.vector.tensor_copy(
                        v_sb[:, (4 * s + u) * 128:(4 * s + u + 1) * 128], tpsum)

                # Q^T tiles 2s, 2s+1 (scale folded in)
                for half in range(2):
                    tq = 2 * s + half
                    qpsum = pproj.tile([128, QG], F32, tag="proj")
                    for c in range(NCH):
                        nc.tensor.matmul(
                            qpsum, lhsT=w_sb["wq"][:, c],
                            rhs=xs[c][:, half * QG:(half + 1) * QG],
                            start=(c == 0), stop=(c == NCH - 1))
                    nc.vector.tensor_copy(qT[:, tq * QG:(tq + 1) * QG], qpsum)

                # attention for the two groups unlocked by this supertile
                attention_group(2 * s)
                attention_group(2 * s + 1)

    nc.finalize()
    return nc


def _get_program():
    if "nc" not in _prog_cache:
        _prog_cache["nc"] = _build_program()
    return _prog_cache["nc"]


def _host_prepare(x, Wq, Wk, Wv):
    """Per-core inputs. Core c: batch b=c//2, parity h=c%2."""
    per_core = []
    for c in range(8):
        b, h = c // 2, c % 2
        pos2glob = np.arange(NKB)
        if h == 1:
            pos2glob = pos2glob.reshape(-1, 2)[:, ::-1].reshape(-1)
        perm = (pos2glob[:, None] * KB + np.arange(KB)[None, :]).reshape(-1)
        xtb = np.ascontiguousarray(x[b].T[:, perm])
        sub = np.arange(QG) // KB
        off = np.arange(QG) % KB
        glob_sub = sub if h == 0 else (sub ^ 1)
        qoff = glob_sub * KB + off
        kk = np.arange(KB)[:, None]
        m0 = (qoff[None, :] >= kk + h * KB).astype(np.float32)
        m1 = (qoff[None, :] >= kk + h * KB + 256).astype(np.float32)
        per_core.append(dict(perm=perm, in_map={
            "xt": xtb, "wq": np.asarray(Wq, np.float32) * SCALE,
            "wk": np.asarray(Wk, np.float32), "wv": np.asarray(Wv, np.float32),
            "mp": np.ascontiguousarray(np.concatenate([m0, m1], axis=1)),
        }))
    return per_core


def run(x, Wq, Wk, Wv, trace=False):
    from concourse.bass_utils import run_bass_kernel_spmd

    x = np.asarray(x, np.float32)
    nc = _get_program()
    per_core = _host_prepare(x, Wq, Wk, Wv)
    res = run_bass_kernel_spmd(
        nc, [pc["in_map"] for pc in per_core], core_ids=list(range(8)),
        trace=trace,
    )
    out = np.zeros((B, T, H), np.float32)
    for b in range(B):
        num = np.zeros((H, T), np.float64)
        den = np.zeros((1, T), np.float64)
        for c in (2 * b, 2 * b + 1):
            inv = np.argsort(per_core[c]["perm"])
            num += res.results[c]["ot"][:, inv]
            den += res.results[c]["dn"][:, inv]
        out[b] = (num / den).T
    return out, res


def kernel(x, Wq, Wk, Wv):
    out, _ = run(x, Wq, Wk, Wv, trace=False)
    return out



# revision 9
# speedup vs baseline: 1.2363x; 1.2363x over previous
"""Causal single-head attention (B=4, T=4096, C=1024, H=128) on 8 Trainium2
NeuronCores — bf16 edition.

Sharding: 8 cores = 4 batches x 2 key-parity shards: each core handles one
batch and the keys in every other 128-block (parity h = core % 2), computing
UN-normalized partial attention (numerator O^T and denominator) for ALL 4096
queries of its batch. The host passes x^T with columns block-permuted so this
core's keys sit at the EVEN 128-block positions, plus a pre-gathered x^T
restricted to those key columns (xkv) so K/V projections stream contiguously,
plus the two diagonal-block mask tiles. Host un-permutes the query axis and
combines: out = (O0 + O1) / (d0 + d1).

Numerics: bf16 operands everywhere on the PE (1 cycle/row, FWL hides
LDWEIGHTS), fp32 PSUM accumulation. Measured end-to-end rel err ~5e-3.
fp8 was tried and rejected: each fp8 quantization on the value path alone
contributes 2-4e-2 relative error (no sqrt(N) averaging — the output norm
shrinks at the same rate as the error), busting the 2e-2 budget.

Per query group j (512 queries, nk=2(j+1) key blocks):
  S^T pair p -> one PSUM tile [128, 1024] (two bank-aligned halves, one
  matmul each); exp via one ACT activation [128,1024] PSUM->SBUF bf16 with
  the softmax scale folded in and bias -3; diagonal pair masked by one DVE
  multiply. PV: two bf16 matmuls per pair into the group's O^T PSUM.
  Denominator: a bf16 accumulator [128,1024] on the DVE (copy + nk/2-1 adds,
  all-bf16 operands so the DVE runs in its 2x/4x perf mode), reduced by two
  ones-matmuls into a [1,512] PSUM at group end.
The last pair's PV lags into the next group so the PE never waits on ACT.
"""

import sys
import numpy as np

sys.path.insert(0, "/opt/trn_rl_repo")

B, T, C, H = 4, 4096, 1024, 128
KB = 128            # key block
QG = 512            # query group
NKB = T // KB       # 32 global key blocks
NQG = T // QG       # 8 query groups
NCH = C // 128      # 8 contraction chunks
NST = 4             # supertiles of 1024 positions
SCALE = float(H) ** -0.5
EXPBIAS = -3.0

_prog_cache = {}


def _build_program():
    import concourse.mybir as mybir
    import concourse.tile as tile
    from concourse import bacc
    from concourse.masks import make_identity

    F32 = mybir.dt.float32
    BF16 = mybir.dt.bfloat16
    AF = mybir.ActivationFunctionType

    nc = bacc.Bacc()
    xt = nc.dram_tensor("xt", [C, T], BF16, kind="ExternalInput")
    xkv = nc.dram_tensor("xkv", [C, T // 2], BF16, kind="ExternalInput")
    wq = nc.dram_tensor("wq", [C, H], BF16, kind="ExternalInput")
    wk = nc.dram_tensor("wk", [C, H], BF16, kind="ExternalInput")
    wv = nc.dram_tensor("wv", [C, H], BF16, kind="ExternalInput")
    mp = nc.dram_tensor("mp", [KB, 2 * QG], BF16, kind="ExternalInput")
    ot = nc.dram_tensor("ot", [H, T], F32, kind="ExternalOutput")
    dn = nc.dram_tensor("dn", [1, T], F32, kind="ExternalOutput")

    with tile.TileContext(nc) as tc:
        with (
            tc.tile_pool(name="singles", bufs=1) as singles,
            tc.tile_pool(name="persist", bufs=1) as persist,
            tc.tile_pool(name="xqp", bufs=3) as xqp,
            tc.tile_pool(name="xkvp", bufs=3) as xkvp,
            tc.tile_pool(name="epool", bufs=5) as epool,
            tc.tile_pool(name="eacc", bufs=2) as eaccp,
            tc.tile_pool(name="vstage", bufs=2) as vstagep,
            tc.tile_pool(name="outs", bufs=4) as outsp,
            tc.tile_pool(name="spair", bufs=2, space="PSUM") as spairp,
            tc.tile_pool(name="pop", bufs=1, space="PSUM") as pop,
            tc.tile_pool(name="pproj", bufs=2, space="PSUM") as pproj,
        ):
            # ---- constants ----
            w_sb = {}
            for name, w in (("wq", wq), ("wk", wk), ("wv", wv)):
                t_ = singles.tile([128, NCH, 128], BF16, tag=f"w_{name}")
                nc.sync.dma_start(out=t_, in_=w.rearrange("(c p) h -> p c h", p=128))
                w_sb[name] = t_
            mp_sb = singles.tile([KB, 2 * QG], BF16, tag="mp")
            nc.sync.dma_start(out=mp_sb, in_=mp[:])
            ones1 = singles.tile([128, 1], BF16, tag="ones1")
            nc.vector.memset(ones1, 1.0)
            ebias = singles.tile([128, 1], F32, tag="ebias")
            nc.vector.memset(ebias, EXPBIAS)
            ident_f = singles.tile([128, 128], F32, tag="ident_f")
            make_identity(nc, ident_f)
            identb = singles.tile([128, 128], BF16, tag="identb")
            nc.scalar.activation(identb, ident_f, AF.Copy)

            qT = persist.tile([128, T], BF16, tag="qT")
            kT = persist.tile([128, T // 2], BF16, tag="kT")
            v_sb = persist.tile([128, T // 2], BF16, tag="v")

            state = {}

            def pv(es, p, start, stop):
                if start:
                    state["o"] = pop.tile([128, QG], F32, tag="opsum",
                                          name="opsum")
                for half in range(2):
                    blk = 2 * p + half
                    nc.tensor.matmul(
                        state["o"],
                        lhsT=v_sb[:, blk * KB:(blk + 1) * KB],
                        rhs=es[p][:, half * QG:(half + 1) * QG],
                        start=start and half == 0, stop=stop and half == 1,
                    )

            def attention_group(j, pending):
                npair = j + 1
                qrhs = qT[:, j * QG:(j + 1) * QG]
                es = []
                acc = eaccp.tile([KB, 2 * QG], BF16, tag="eacc")
                for p in range(npair):
                    sp = spairp.tile([KB, 2 * QG], F32, tag="spair")
                    nc.tensor.matmul(
                        sp[:, 0:QG],
                        lhsT=kT[:, 2 * p * KB:(2 * p + 1) * KB],
                        rhs=qrhs, start=True, stop=True,
                    )
                    nc.tensor.matmul(
                        sp[:, QG:2 * QG],
                        lhsT=kT[:, (2 * p + 1) * KB:(2 * p + 2) * KB],
                        rhs=qrhs, start=True, stop=True,
                    )
                    if p == 0 and pending is not None:
                        pending()
                    ep = epool.tile([KB, 2 * QG], BF16, tag="e")
                    nc.scalar.activation(ep, sp, AF.Exp,
                                         bias=ebias, scale=SCALE)
                    if p == npair - 1:
                        nc.vector.tensor_mul(ep, ep, mp_sb)
                    es.append(ep)
                    if p == 0:
                        nc.vector.tensor_copy(acc, ep)
                    else:
                        nc.vector.tensor_add(acc, acc, ep)
                    if p >= 1:
                        pv(es, p - 1, start=(p == 1), stop=False)

                def flush():
                    pv(es, npair - 1, start=(npair == 1), stop=True)
                    dps = pop.tile([1, QG], F32, tag="dpsum", name="dpsum")
                    nc.tensor.matmul(dps, lhsT=ones1, rhs=acc[:, 0:QG],
                                     start=True, stop=False)
                    nc.tensor.matmul(dps, lhsT=ones1, rhs=acc[:, QG:2 * QG],
                                     start=False, stop=True)
                    osb = outsp.tile([128, QG], F32, tag="osb")
                    nc.vector.tensor_copy(osb, state["o"])
                    nc.sync.dma_start(out=ot[:, j * QG:(j + 1) * QG], in_=osb)
                    dsb = outsp.tile([1, QG], F32, tag="dsb")
                    nc.vector.tensor_copy(dsb, dps)
                    nc.sync.dma_start(out=dn[:, j * QG:(j + 1) * QG], in_=dsb)
                return flush

            pending = None
            for s in range(NST):
                xq = xqp.tile([128, NCH * 1024], BF16, tag="xq")
                nc.sync.dma_start(
                    out=xq.rearrange("p (c t) -> p c t", c=NCH),
                    in_=xt[:, s * 1024:(s + 1) * 1024].rearrange(
                        "(c p) t -> p c t", p=128),
                )
                xkv_t = xkvp.tile([128, NCH * 512], BF16, tag="xkv")
                nc.sync.dma_start(
                    out=xkv_t.rearrange("p (c t) -> p c t", c=NCH),
                    in_=xkv[:, s * 512:(s + 1) * 512].rearrange(
                        "(c p) t -> p c t", p=128),
                )
                xqv = xq.rearrange("p (c t) -> p c t", c=NCH)
                xkvv = xkv_t.rearrange("p (c t) -> p c t", c=NCH)

                # K^T tile s [128, 512] (bf16)
                kp = pproj.tile([128, QG], F32, tag="proj")
                for c in range(NCH):
                    nc.tensor.matmul(
                        kp, lhsT=w_sb["wk"][:, c, :], rhs=xkvv[:, c, :],
                        start=(c == 0), stop=(c == NCH - 1),
                    )
                nc.vector.tensor_copy(kT[:, s * QG:(s + 1) * QG], kp)

                # V^T tile s -> bf16 -> transpose into V rows [key, h]
                vp = pproj.tile([128, QG], F32, tag="proj")
                for c in range(NCH):
                    nc.tensor.matmul(
                        vp, lhsT=w_sb["wv"][:, c, :], rhs=xkvv[:, c, :],
                        start=(c == 0), stop=(c == NCH - 1),
                    )
                vstage = vstagep.tile([128, QG], BF16, tag="vstage")
                nc.vector.tensor_copy(vstage, vp)
                for u in range(4):
                    tp = pproj.tile([128, 128], BF16, tag="proj")
                    nc.tensor.transpose(
                        tp, vstage[:, u * 128:(u + 1) * 128], identb)
                    nc.vector.tensor_copy(
                        v_sb[:, (4 * s + u) * 128:(4 * s + u + 1) * 128], tp)

                # Q^T tiles 2s, 2s+1 (bf16)
                for half in range(2):
                    tq = 2 * s + half
                    qp = pproj.tile([128, QG], F32, tag="proj")
                    for c in range(NCH):
                        nc.tensor.matmul(
                            qp, lhsT=w_sb["wq"][:, c, :],
                            rhs=xqv[:, c, half * QG:(half + 1) * QG],
                            start=(c == 0), stop=(c == NCH - 1),
                        )
                    nc.vector.tensor_copy(qT[:, tq * QG:(tq + 1) * QG], qp)

                pending = attention_group(2 * s, pending)
                pending = attention_group(2 * s + 1, pending)
            pending()

    nc.finalize()
    return nc


def _get_program():
    if "nc" not in _prog_cache:
        _prog_cache["nc"] = _build_program()
    return _prog_cache["nc"]


def _to_bf16(a):
    import ml_dtypes
    return np.asarray(a, np.float32).astype(ml_dtypes.bfloat16)


def _host_prepare(x, Wq, Wk, Wv):
    """Per-core inputs. Core c: batch b=c//2, parity h=c%2."""
    w16 = {n: _to_bf16(w) for n, w in (("wq", Wq), ("wk", Wk), ("wv", Wv))}
    per_core = []
    for c in range(8):
        b, h = c // 2, c % 2
        pos2glob = np.arange(NKB)
        if h == 1:
            pos2glob = pos2glob.reshape(-1, 2)[:, ::-1].reshape(-1)
        perm = (pos2glob[:, None] * KB + np.arange(KB)[None, :]).reshape(-1)
        xtb = _to_bf16(x[b].T[:, perm])
        xkvb = np.ascontiguousarray(
            xtb.reshape(C, NKB // 2, 2, KB)[:, :, 0, :].reshape(C, T // 2))
        sub = np.arange(QG) // KB
        off = np.arange(QG) % KB
        glob_sub = sub if h == 0 else (sub ^ 1)
        qoff = glob_sub * KB + off
        kk = np.arange(KB)[:, None]
        m0 = (qoff[None, :] >= kk + h * KB).astype(np.float32)
        m1 = (qoff[None, :] >= kk + h * KB + 256).astype(np.float32)
        per_core.append(dict(perm=perm, in_map={
            "xt": np.ascontiguousarray(xtb),
            "xkv": xkvb,
            "wq": w16["wq"], "wk": w16["wk"], "wv": w16["wv"],
            "mp": _to_bf16(np.concatenate([m0, m1], axis=1)),
        }))
    return per_core


def run(x, Wq, Wk, Wv, trace=False):
    from concourse.bass_utils import run_bass_kernel_spmd

    x = np.asarray(x, np.float32)
    nc = _get_program()
    per_core = _host_prepare(x, Wq, Wk, Wv)
    res = run_bass_kernel_spmd(
        nc, [pc["in_map"] for pc in per_core], core_ids=list(range(8)),
        trace=trace,
    )
    out = np.zeros((B, T, H), np.float32)
    for b in range(B):
        num = np.zeros((H, T), np.float64)
        den = np.zeros((1, T), np.float64)
        for c in (2 * b, 2 * b + 1):
            inv = np.argsort(per_core[c]["perm"])
            num += res.results[c]["ot"][:, inv]
            den += res.results[c]["dn"][:, inv]
        out[b] = (num / den).T
    return out, res


def kernel(x, Wq, Wk, Wv):
    out, _ = run(x, Wq, Wk, Wv, trace=False)
    return out


# revision 10
# speedup vs baseline: 1.4359x; 1.1614x over previous
"""Causal single-head attention (B=4, T=4096, C=1024, H=128) on 8 Trainium2
NeuronCores — bf16, pipelined-startup edition.

Sharding: 8 cores = 4 batches x 2 key-parity shards: each core handles one
batch and the keys in every other 128-block (parity h = core % 2), computing
UN-normalized partial attention (numerator O^T and denominator) for ALL 4096
queries of its batch. The host passes x^T with columns block-permuted so this
core's keys sit at the EVEN 128-block positions, plus a pre-gathered x^T
restricted to those key columns (xkv), plus the two diagonal-block mask
tiles, plus weights already in the on-chip [128, c*128] layout so their DMA
is contiguous. Host un-permutes the query axis and combines:
out = (O0 + O1) / (d0 + d1).

Numerics: bf16 operands everywhere on the PE (1 cycle/row, FWL hides
LDWEIGHTS), fp32 PSUM accumulation. Measured end-to-end rel err ~5e-3.
fp8 was tried and rejected: each fp8 quantization on the value path alone
contributes 2-4e-2 relative error, busting the 2e-2 budget.

Pipelining:
  - x streams on the gpsimd DMA queue, weights/masks/outputs on the sync
    queue, so input streaming and output drain don't serialize.
  - Supertile 0's x tiles arrive as per-chunk DMAs so the first projection
    matmul only waits for chunk 0, not the whole megatile.
  - ~24 tiny warmup matmuls run during the DMA fill to lift the PE out of
    its cold HAM state before real work arrives.
  - Per query group j (512 queries, nk=2(j+1) key blocks): S^T pair ->
    PSUM [128,1024]; exp via one ACT activation -> bf16 SBUF with softmax
    scale + bias -3 folded in; diagonal pair masked by one DVE multiply;
    PV lags TWO pairs behind S/exp (the last two pairs spill into the next
    group) so the PE never waits on ACT; denominator accumulates on the DVE
    (all-bf16 2x mode) and reduces with two ones-matmuls per group.
"""

import sys
import numpy as np

sys.path.insert(0, "/opt/trn_rl_repo")

B, T, C, H = 4, 4096, 1024, 128
KB = 128            # key block
QG = 512            # query group
NKB = T // KB       # 32 global key blocks
NQG = T // QG       # 8 query groups
NCH = C // 128      # 8 contraction chunks
NST = 4             # supertiles of 1024 positions
SCALE = float(H) ** -0.5
EXPBIAS = -3.0

_prog_cache = {}


def _build_program():
    import concourse.mybir as mybir
    import concourse.tile as tile
    from concourse import bacc
    from concourse.masks import make_identity

    F32 = mybir.dt.float32
    BF16 = mybir.dt.bfloat16
    AF = mybir.ActivationFunctionType

    nc = bacc.Bacc()
    xt = nc.dram_tensor("xt", [C, T], BF16, kind="ExternalInput")
    xkv = nc.dram_tensor("xkv", [C, T // 2], BF16, kind="ExternalInput")
    wq = nc.dram_tensor("wq", [128, NCH * 128], BF16, kind="ExternalInput")
    wk = nc.dram_tensor("wk", [128, NCH * 128], BF16, kind="ExternalInput")
    wv = nc.dram_tensor("wv", [128, NCH * 128], BF16, kind="ExternalInput")
    mp = nc.dram_tensor("mp", [KB, 2 * QG], BF16, kind="ExternalInput")
    ot = nc.dram_tensor("ot", [H, T], F32, kind="ExternalOutput")
    dn = nc.dram_tensor("dn", [1, T], F32, kind="ExternalOutput")

    with tile.TileContext(nc) as tc:
        with (
            tc.tile_pool(name="singles", bufs=1) as singles,
            tc.tile_pool(name="persist", bufs=1) as persist,
            tc.tile_pool(name="xqp", bufs=3) as xqp,
            tc.tile_pool(name="xkvp", bufs=3) as xkvp,
            tc.tile_pool(name="epool", bufs=6) as epool,
            tc.tile_pool(name="eacc", bufs=2) as eaccp,
            tc.tile_pool(name="vstage", bufs=2) as vstagep,
            tc.tile_pool(name="outs", bufs=4) as outsp,
            tc.tile_pool(name="spair", bufs=2, space="PSUM") as spairp,
            tc.tile_pool(name="pop", bufs=1, space="PSUM") as pop,
            tc.tile_pool(name="pproj", bufs=2, space="PSUM") as pproj,
        ):
            # ---- constants (DVE memsets run during the engine preamble) ----
            scratch = singles.tile([128, 128], BF16, tag="scratch")
            nc.vector.memset(scratch, 0.125)
            ones1 = singles.tile([128, 1], BF16, tag="ones1")
            nc.vector.memset(ones1, 1.0)
            ebias = singles.tile([128, 1], F32, tag="ebias")
            nc.vector.memset(ebias, EXPBIAS)

            # ---- PE warmup: lift HAM out of the cold state during DMA fill
            wps = pop.tile([1, QG], F32, tag="dpsum", name="wpsum")
            for _ in range(24):
                nc.tensor.matmul(wps[:, 0:128], lhsT=scratch[:, 0:1],
                                 rhs=scratch, start=True, stop=True)

            # ---- weights (sync queue), masks; x (gpsimd queue) ----
            mp_sb = singles.tile([KB, 2 * QG], BF16, tag="mp")
            nc.sync.dma_start(out=mp_sb, in_=mp[:])
            w_sb = {}
            for name, w in (("wk", wk), ("wv", wv), ("wq", wq)):
                t_ = singles.tile([128, NCH * 128], BF16, tag=f"w_{name}")
                nc.sync.dma_start(out=t_, in_=w[:])
                w_sb[name] = t_.rearrange("p (c h) -> p c h", c=NCH)
            ident_f = singles.tile([128, 128], F32, tag="ident_f")
            make_identity(nc, ident_f)
            identb = singles.tile([128, 128], BF16, tag="identb")
            nc.scalar.activation(identb, ident_f, AF.Copy)

            qT = persist.tile([128, T], BF16, tag="qT")
            kT = persist.tile([128, T // 2], BF16, tag="kT")
            v_sb = persist.tile([128, T // 2], BF16, tag="v")

            def stream_x(s):
                xq = xqp.tile([128, NCH * 1024], BF16, tag="xq")
                xkv_t = xkvp.tile([128, NCH * 512], BF16, tag="xkv")
                if s == 0:
                    for c in range(NCH):
                        nc.gpsimd.dma_start(
                            out=xkv_t[:, c * 512:(c + 1) * 512],
                            in_=xkv[c * 128:(c + 1) * 128, 0:512])
                    for c in range(NCH):
                        nc.gpsimd.dma_start(
                            out=xq[:, c * 1024:(c + 1) * 1024],
                            in_=xt[c * 128:(c + 1) * 128, 0:1024])
                else:
                    nc.gpsimd.dma_start(
                        out=xkv_t.rearrange("p (c t) -> p c t", c=NCH),
                        in_=xkv[:, s * 512:(s + 1) * 512].rearrange(
                            "(c p) t -> p c t", p=128))
                    nc.gpsimd.dma_start(
                        out=xq.rearrange("p (c t) -> p c t", c=NCH),
                        in_=xt[:, s * 1024:(s + 1) * 1024].rearrange(
                            "(c p) t -> p c t", p=128))
                return (xq.rearrange("p (c t) -> p c t", c=NCH),
                        xkv_t.rearrange("p (c t) -> p c t", c=NCH))

            def proj_steps(s, xqv, xkvv):
                def kproj():
                    kp = pproj.tile([128, QG], F32, tag="proj", name="kp")
                    for c in range(NCH):
                        nc.tensor.matmul(
                            kp, lhsT=w_sb["wk"][:, c, :], rhs=xkvv[:, c, :],
                            start=(c == 0), stop=(c == NCH - 1))
                    nc.vector.tensor_copy(kT[:, s * QG:(s + 1) * QG], kp)

                vstage = vstagep.tile([128, QG], BF16, tag="vstage")

                def vproj():
                    vp = pproj.tile([128, QG], F32, tag="proj", name="vp")
                    for c in range(NCH):
                        nc.tensor.matmul(
                            vp, lhsT=w_sb["wv"][:, c, :], rhs=xkvv[:, c, :],
                            start=(c == 0), stop=(c == NCH - 1))
                    nc.vector.tensor_copy(vstage, vp)

                def qproj(half):
                    def f():
                        qp = pproj.tile([128, QG], F32, tag="proj", name="qp")
                        for c in range(NCH):
                            nc.tensor.matmul(
                                qp, lhsT=w_sb["wq"][:, c, :],
                                rhs=xqv[:, c, half * QG:(half + 1) * QG],
                                start=(c == 0), stop=(c == NCH - 1))
                        nc.vector.tensor_copy(
                            qT[:, (2 * s + half) * QG:
                               (2 * s + half + 1) * QG], qp)
                    return f

                def vtrans():
                    for u in range(4):
                        tp = pproj.tile([128, 128], BF16, tag="proj",
                                        name="tp")
                        nc.tensor.transpose(
                            tp, vstage[:, u * 128:(u + 1) * 128], identb)
                        nc.vector.tensor_copy(
                            v_sb[:, (4 * s + u) * 128:(4 * s + u + 1) * 128],
                            tp)

                return [kproj, vproj, qproj(0), qproj(1), vtrans]

            state = {}

            def pv(es, p, start, stop):
                if start:
                    state["o"] = pop.tile([128, QG], F32, tag="opsum",
                                          name="opsum")
                for half in range(2):
                    blk = 2 * p + half
                    nc.tensor.matmul(
                        state["o"],
                        lhsT=v_sb[:, blk * KB:(blk + 1) * KB],
                        rhs=es[p][:, half * QG:(half + 1) * QG],
                        start=start and half == 0, stop=stop and half == 1)

            def attention_group(j, pending, interleave=()):
                npair = j + 1
                qrhs = qT[:, j * QG:(j + 1) * QG]
                es = []
                acc = eaccp.tile([KB, 2 * QG], BF16, tag="eacc", name="eacc")
                inter = list(interleave)
                for p in range(npair):
                    sp = spairp.tile([KB, 2 * QG], F32, tag="spair",
                                     name="sp")
                    nc.tensor.matmul(
                        sp[:, 0:QG],
                        lhsT=kT[:, 2 * p * KB:(2 * p + 1) * KB],
                        rhs=qrhs, start=True, stop=True)
                    nc.tensor.matmul(
                        sp[:, QG:2 * QG],
                        lhsT=kT[:, (2 * p + 1) * KB:(2 * p + 2) * KB],
                        rhs=qrhs, start=True, stop=True)
                    if p == 0 and pending is not None:
                        pending()
                    elif p >= 1 and inter:
                        inter.pop(0)()
                    ep = epool.tile([KB, 2 * QG], BF16, tag="e", name="ep")
                    nc.scalar.activation(ep, sp, AF.Exp,
                                         bias=ebias, scale=SCALE)
                    if p == npair - 1:
                        nc.vector.tensor_mul(ep, ep, mp_sb)
                    es.append(ep)
                    if p == 0:
                        nc.vector.tensor_copy(acc, ep)
                    else:
                        nc.vector.tensor_add(acc, acc, ep)
                    if p >= 2:
                        pv(es, p - 2, start=(p == 2), stop=False)
                for f in inter:
                    f()

                def flush():
                    for pp in (npair - 2, npair - 1):
                        if pp >= 0:
                            pv(es, pp, start=(pp == 0), stop=(pp == npair - 1))
                    dps = pop.tile([1, QG], F32, tag="dpsum", name="dpsum")
                    nc.tensor.matmul(dps, lhsT=ones1, rhs=acc[:, 0:QG],
                                     start=True, stop=False)
                    nc.tensor.matmul(dps, lhsT=ones1, rhs=acc[:, QG:2 * QG],
                                     start=False, stop=True)
                    osb = outsp.tile([128, QG], F32, tag="osb")
                    nc.vector.tensor_copy(osb, state["o"])
                    nc.sync.dma_start(out=ot[:, j * QG:(j + 1) * QG], in_=osb)
                    dsb = outsp.tile([1, QG], F32, tag="dsb")
                    nc.vector.tensor_copy(dsb, dps)
                    nc.sync.dma_start(out=dn[:, j * QG:(j + 1) * QG], in_=dsb)
                return flush

            pending = None
            xqv, xkvv = stream_x(0)
            for f in proj_steps(0, xqv, xkvv):
                f()
            for s in range(NST):
                if s + 1 < NST:
                    nxq, nxkv = stream_x(s + 1)
                    nxt = proj_steps(s + 1, nxq, nxkv)
                else:
                    nxt = []
                pending = attention_group(2 * s, pending)
                pending = attention_group(2 * s + 1, pending, interleave=nxt)
            pending()

    nc.finalize()
    return nc


def _get_program():
    if "nc" not in _prog_cache:
        _prog_cache["nc"] = _build_program()
    return _prog_cache["nc"]


def _to_bf16(a):
    import ml_dtypes
    return np.asarray(a, np.float32).astype(ml_dtypes.bfloat16)


def _warr(w):
    # host-side rearrange to the on-chip [128, c*128] layout (contiguous DMA)
    return np.ascontiguousarray(
        _to_bf16(w).reshape(NCH, 128, H).transpose(1, 0, 2).reshape(128, -1))


def _host_prepare(x, Wq, Wk, Wv):
    """Per-core inputs. Core c: batch b=c//2, parity h=c%2."""
    w16 = {n: _warr(w) for n, w in (("wq", Wq), ("wk", Wk), ("wv", Wv))}
    per_core = []
    for c in range(8):
        b, h = c // 2, c % 2
        pos2glob = np.arange(NKB)
        if h == 1:
            pos2glob = pos2glob.reshape(-1, 2)[:, ::-1].reshape(-1)
        perm = (pos2glob[:, None] * KB + np.arange(KB)[None, :]).reshape(-1)
        xtb = _to_bf16(x[b].T[:, perm])
        xkvb = np.ascontiguousarray(
            xtb.reshape(C, NKB // 2, 2, KB)[:, :, 0, :].reshape(C, T // 2))
        sub = np.arange(QG) // KB
        off = np.arange(QG) % KB
        glob_sub = sub if h == 0 else (sub ^ 1)
        qoff = glob_sub * KB + off
        kk = np.arange(KB)[:, None]
        m0 = (qoff[None, :] >= kk + h * KB).astype(np.float32)
        m1 = (qoff[None, :] >= kk + h * KB + 256).astype(np.float32)
        per_core.append(dict(perm=perm, in_map={
            "xt": np.ascontiguousarray(xtb),
            "xkv": xkvb,
            "wq": w16["wq"], "wk": w16["wk"], "wv": w16["wv"],
            "mp": _to_bf16(np.concatenate([m0, m1], axis=1)),
        }))
    return per_core


def run(x, Wq, Wk, Wv, trace=False):
    from concourse.bass_utils import run_bass_kernel_spmd

    x = np.asarray(x, np.float32)
    nc = _get_program()
    per_core = _host_prepare(x, Wq, Wk, Wv)
    res = run_bass_kernel_spmd(
        nc, [pc["in_map"] for pc in per_core], core_ids=list(range(8)),
        trace=trace,
    )
    out = np.zeros((B, T, H), np.float32)
    for b in range(B):
        num = np.zeros((H, T), np.float64)
        den = np.zeros((1, T), np.float64)
        for c in (2 * b, 2 * b + 1):
            inv = np.argsort(per_core[c]["perm"])
            num += res.results[c]["ot"][:, inv]
            den += res.results[c]["dn"][:, inv]
        out[b] = (num / den).T
    return out, res


def kernel(x, Wq, Wk, Wv):
    out, _ = run(x, Wq, Wk, Wv, trace=False)
    return out


# revision 13
# speedup vs baseline: 1.4615x; 1.0178x over previous
"""Causal single-head attention (B=4, T=4096, C=1024, H=128) on 8 Trainium2
NeuronCores — bf16, pipelined-startup edition.

Sharding: 8 cores = 4 batches x 2 key-parity shards: each core handles one
batch and the keys in every other 128-block (parity h = core % 2), computing
UN-normalized partial attention (numerator O^T and denominator) for ALL 4096
queries of its batch. The host passes x^T with columns block-permuted so this
core's keys sit at the EVEN 128-block positions, plus a pre-gathered x^T
restricted to those key columns (xkv), plus the two diagonal-block mask
tiles, plus weights already in the on-chip [128, c*128] layout so their DMA
is contiguous. Host un-permutes the query axis and combines:
out = (O0 + O1) / (d0 + d1).

Numerics: bf16 operands everywhere on the PE (1 cycle/row, FWL hides
LDWEIGHTS), fp32 PSUM accumulation. Measured end-to-end rel err ~5e-3.
fp8 was tried and rejected: each fp8 quantization on the value path alone
contributes 2-4e-2 relative error, busting the 2e-2 budget.

Pipelining:
  - x streams on the gpsimd DMA queue, weights/masks/outputs on the sync
    queue, so input streaming and output drain don't serialize.
  - Supertile 0's x tiles arrive as per-chunk DMAs so the first projection
    matmul only waits for chunk 0, not the whole megatile.
  - ~24 tiny warmup matmuls run during the DMA fill to lift the PE out of
    its cold HAM state before real work arrives.
  - Per query group j (512 queries, nk=2(j+1) key blocks): S^T pair ->
    PSUM [128,1024]; exp via one ACT activation -> bf16 SBUF with softmax
    scale + bias -3 folded in; diagonal pair masked by one DVE multiply;
    PV lags TWO pairs behind S/exp (the last two pairs spill into the next
    group) so the PE never waits on ACT; denominator accumulates on the DVE
    (all-bf16 2x mode) and reduces with two ones-matmuls per group.
"""

import sys
import numpy as np

sys.path.insert(0, "/opt/trn_rl_repo")

B, T, C, H = 4, 4096, 1024, 128
KB = 128            # key block
QG = 512            # query group
NKB = T // KB       # 32 global key blocks
NQG = T // QG       # 8 query groups
NCH = C // 128      # 8 contraction chunks
NST = 4             # supertiles of 1024 positions
SCALE = float(H) ** -0.5
EXPBIAS = -3.0

_prog_cache = {}


def _build_program():
    import concourse.mybir as mybir
    import concourse.tile as tile
    from concourse import bacc

    F32 = mybir.dt.float32
    BF16 = mybir.dt.bfloat16
    AF = mybir.ActivationFunctionType

    nc = bacc.Bacc()
    xt = nc.dram_tensor("xt", [128, NST * NCH * 1024], BF16,
                        kind="ExternalInput")
    wq = nc.dram_tensor("wq", [128, NCH * 128], BF16, kind="ExternalInput")
    wk = nc.dram_tensor("wk", [128, NCH * 128], BF16, kind="ExternalInput")
    wv = nc.dram_tensor("wv", [128, NCH * 128], BF16, kind="ExternalInput")
    mp = nc.dram_tensor("mp", [KB, 2 * QG], BF16, kind="ExternalInput")
    idb = nc.dram_tensor("idb", [128, 128], BF16, kind="ExternalInput")
    ot = nc.dram_tensor("ot", [H, T], BF16, kind="ExternalOutput")
    dn = nc.dram_tensor("dn", [1, T], F32, kind="ExternalOutput")

    with tile.TileContext(nc) as tc:
        with (
            tc.tile_pool(name="singles", bufs=1) as singles,
            tc.tile_pool(name="persist", bufs=1) as persist,
            tc.tile_pool(name="xqp", bufs=4) as xqp,
            tc.tile_pool(name="epool", bufs=6) as epool,
            tc.tile_pool(name="eacc", bufs=2) as eaccp,
            tc.tile_pool(name="vstage", bufs=2) as vstagep,
            tc.tile_pool(name="outs", bufs=4) as outsp,
            tc.tile_pool(name="spair", bufs=2, space="PSUM") as spairp,
            tc.tile_pool(name="pop", bufs=1, space="PSUM") as pop,
            tc.tile_pool(name="pproj", bufs=2, space="PSUM") as pproj,
        ):
            # ---- constants (DVE memsets run during the engine preamble) ----
            scratch = singles.tile([128, QG], BF16, tag="scratch")
            nc.vector.memset(scratch, 0.125)
            ones1 = singles.tile([128, 1], BF16, tag="ones1")
            nc.vector.memset(ones1, 1.0)
            ebias = singles.tile([128, 1], F32, tag="ebias")
            nc.vector.memset(ebias, EXPBIAS)

            # ---- PE warmup: lift HAM out of the cold state during DMA fill
            wps = pop.tile([1, QG], F32, tag="dpsum", name="wpsum")
            for _ in range(6):
                nc.tensor.matmul(wps, lhsT=scratch[:, 0:1],
                                 rhs=scratch, start=True, stop=True)

            # ---- weights (sync queue), masks; x (gpsimd queue) ----
            mp_sb = singles.tile([KB, 2 * QG], BF16, tag="mp")
            nc.sync.dma_start(out=mp_sb, in_=mp[:])
            w_sb = {}
            for name, w in (("wk", wk), ("wv", wv), ("wq", wq)):
                t_ = singles.tile([128, NCH * 128], BF16, tag=f"w_{name}")
                nc.sync.dma_start(out=t_, in_=w[:])
                w_sb[name] = t_.rearrange("p (c h) -> p c h", c=NCH)
            identb = singles.tile([128, 128], BF16, tag="identb")
            nc.sync.dma_start(out=identb, in_=idb[:])

            qT = persist.tile([128, T], BF16, tag="qT")
            kT = persist.tile([128, T // 2], BF16, tag="kT")
            v_sb = persist.tile([128, T // 2], BF16, tag="v")

            def stream_x(s):
                xq = xqp.tile([128, NCH * 1024], BF16, tag="xq")
                base = s * NCH * 1024
                if s == 0:
                    for c_ in range(NCH):
                        nc.gpsimd.dma_start(
                            out=xq[:, c_ * 1024:(c_ + 1) * 1024],
                            in_=xt[:, base + c_ * 1024:
                                   base + (c_ + 1) * 1024])
                else:
                    nc.gpsimd.dma_start(
                        out=xq, in_=xt[:, base:base + NCH * 1024])
                # full view and even-key-block view (K/V read the latter)
                return (xq.rearrange("p (c t) -> p c t", c=NCH),
                        xq.rearrange("p (c u par b) -> p c u par b",
                                     c=NCH, u=4, par=2, b=128))

            def proj_steps(s, xqv, xkvv):
                def kproj():
                    kp = pproj.tile([128, QG], F32, tag="proj", name="kp")
                    for c in range(NCH):
                        nc.tensor.matmul(
                            kp, lhsT=w_sb["wk"][:, c, :],
                            rhs=xkvv[:, c, :, 0, :],
                            start=(c == 0), stop=(c == NCH - 1))
                    nc.vector.tensor_copy(kT[:, s * QG:(s + 1) * QG], kp)

                vstage = vstagep.tile([128, QG], BF16, tag="vstage")

                def vproj():
                    vp = pproj.tile([128, QG], F32, tag="proj", name="vp")
                    for c in range(NCH):
                        nc.tensor.matmul(
                            vp, lhsT=w_sb["wv"][:, c, :],
                            rhs=xkvv[:, c, :, 0, :],
                            start=(c == 0), stop=(c == NCH - 1))
                    nc.vector.tensor_copy(vstage, vp)

                def qproj(half):
                    def f():
                        qp = pproj.tile([128, QG], F32, tag="proj", name="qp")
                        for c in range(NCH):
                            nc.tensor.matmul(
                                qp, lhsT=w_sb["wq"][:, c, :],
                                rhs=xqv[:, c, half * QG:(half + 1) * QG],
                                start=(c == 0), stop=(c == NCH - 1))
                        nc.vector.tensor_copy(
                            qT[:, (2 * s + half) * QG:
                               (2 * s + half + 1) * QG], qp)
                    return f

                def vtrans():
                    for u in range(4):
                        tp = pproj.tile([128, 128], BF16, tag="proj",
                                        name="tp")
                        nc.tensor.transpose(
                            tp, vstage[:, u * 128:(u + 1) * 128], identb)
                        nc.vector.tensor_copy(
                            v_sb[:, (4 * s + u) * 128:(4 * s + u + 1) * 128],
                            tp)

                return [kproj, vproj, qproj(0), qproj(1), vtrans]

            state = {}

            def pv(es, p, start, stop):
                if start:
                    state["o"] = pop.tile([128, QG], F32, tag="opsum",
                                          name="opsum")
                for half in range(2):
                    blk = 2 * p + half
                    nc.tensor.matmul(
                        state["o"],
                        lhsT=v_sb[:, blk * KB:(blk + 1) * KB],
                        rhs=es[p][:, half * QG:(half + 1) * QG],
                        start=start and half == 0, stop=stop and half == 1)

            def attention_group(j, pending, interleave=()):
                npair = j + 1
                qrhs = qT[:, j * QG:(j + 1) * QG]
                es = []
                acc = eaccp.tile([KB, 2 * QG], BF16, tag="eacc", name="eacc")
                inter = list(interleave)
                for p in range(npair):
                    sp = spairp.tile([KB, 2 * QG], F32, tag="spair",
                                     name="sp")
                    nc.tensor.matmul(
                        sp[:, 0:QG],
                        lhsT=kT[:, 2 * p * KB:(2 * p + 1) * KB],
                        rhs=qrhs, start=True, stop=True)
                    nc.tensor.matmul(
                        sp[:, QG:2 * QG],
                        lhsT=kT[:, (2 * p + 1) * KB:(2 * p + 2) * KB],
                        rhs=qrhs, start=True, stop=True)
                    if p == 0 and pending is not None:
                        pending()
                    elif p >= 1 and inter:
                        inter.pop(0)()
                    ep = epool.tile([KB, 2 * QG], BF16, tag="e", name="ep")
                    nc.scalar.activation(ep, sp, AF.Exp,
                                         bias=ebias, scale=SCALE)
                    if p == npair - 1:
                        nc.vector.tensor_mul(ep, ep, mp_sb)
                    es.append(ep)
                    if p == 0:
                        nc.vector.tensor_copy(acc, ep)
                    else:
                        nc.vector.tensor_add(acc, acc, ep)
                    if p >= 2:
                        pv(es, p - 2, start=(p == 2), stop=False)
                for f in inter:
                    f()

                def flush():
                    for pp in (npair - 2, npair - 1):
                        if pp >= 0:
                            pv(es, pp, start=(pp == 0), stop=(pp == npair - 1))
                    dps = pop.tile([1, QG], F32, tag="dpsum", name="dpsum")
                    nc.tensor.matmul(dps, lhsT=ones1, rhs=acc[:, 0:QG],
                                     start=True, stop=False)
                    nc.tensor.matmul(dps, lhsT=ones1, rhs=acc[:, QG:2 * QG],
                                     start=False, stop=True)
                    osb = outsp.tile([128, QG], BF16, tag="osb")
                    nc.vector.tensor_copy(osb, state["o"])
                    nc.sync.dma_start(out=ot[:, j * QG:(j + 1) * QG], in_=osb)
                    dsb = outsp.tile([1, QG], F32, tag="dsb")
                    nc.vector.tensor_copy(dsb, dps)
                    nc.sync.dma_start(out=dn[:, j * QG:(j + 1) * QG], in_=dsb)
                return flush

            pending = None
            xqv, xkvv = stream_x(0)
            for f in proj_steps(0, xqv, xkvv):
                f()
            for s in range(NST):
                if s + 1 < NST:
                    nxq, nxkv = stream_x(s + 1)
                    nxt = proj_steps(s + 1, nxq, nxkv)
                else:
                    nxt = []
                pending = attention_group(2 * s, pending)
                pending = attention_group(2 * s + 1, pending, interleave=nxt)
            pending()

    nc.finalize()
    return nc


def _get_program():
    if "nc" not in _prog_cache:
        _prog_cache["nc"] = _build_program()
    return _prog_cache["nc"]


def _to_bf16(a):
    import ml_dtypes
    return np.asarray(a, np.float32).astype(ml_dtypes.bfloat16)


def _warr(w):
    # host-side rearrange to the on-chip [128, c*128] layout (contiguous DMA)
    return np.ascontiguousarray(
        _to_bf16(w).reshape(NCH, 128, H).transpose(1, 0, 2).reshape(128, -1))


def _host_prepare(x, Wq, Wk, Wv):
    """Per-core inputs. Core c: batch b=c//2, parity h=c%2."""
    w16 = {n: _warr(w) for n, w in (("wq", Wq), ("wk", Wk), ("wv", Wv))}
    per_core = []
    for c in range(8):
        b, h = c // 2, c % 2
        pos2glob = np.arange(NKB)
        if h == 1:
            pos2glob = pos2glob.reshape(-1, 2)[:, ::-1].reshape(-1)
        perm = (pos2glob[:, None] * KB + np.arange(KB)[None, :]).reshape(-1)
        xtb = _to_bf16(x[b].T[:, perm])
        # [c*128+p, s*1024+t] -> [p, (s c t)] so supertile DMAs are contiguous
        xt2 = np.ascontiguousarray(
            xtb.reshape(NCH, 128, NST, 1024).transpose(1, 2, 0, 3)
            .reshape(128, NST * NCH * 1024))
        sub = np.arange(QG) // KB
        off = np.arange(QG) % KB
        glob_sub = sub if h == 0 else (sub ^ 1)
        qoff = glob_sub * KB + off
        kk = np.arange(KB)[:, None]
        m0 = (qoff[None, :] >= kk + h * KB).astype(np.float32)
        m1 = (qoff[None, :] >= kk + h * KB + 256).astype(np.float32)
        per_core.append(dict(perm=perm, in_map={
            "xt": xt2,
            "wq": w16["wq"], "wk": w16["wk"], "wv": w16["wv"],
            "mp": _to_bf16(np.concatenate([m0, m1], axis=1)),
            "idb": _to_bf16(np.eye(128, dtype=np.float32)),
        }))
    return per_core


def run(x, Wq, Wk, Wv, trace=False):
    from concourse.bass_utils import run_bass_kernel_spmd

    x = np.asarray(x, np.float32)
    nc = _get_program()
    per_core = _host_prepare(x, Wq, Wk, Wv)
    res = run_bass_kernel_spmd(
        nc, [pc["in_map"] for pc in per_core], core_ids=list(range(8)),
        trace=trace,
    )
    out = np.zeros((B, T, H), np.float32)
    for b in range(B):
        num = np.zeros((H, T), np.float64)
        den = np.zeros((1, T), np.float64)
        for c in (2 * b, 2 * b + 1):
            inv = np.argsort(per_core[c]["perm"])
            num += np.asarray(res.results[c]["ot"], np.float32)[:, inv]
            den += res.results[c]["dn"][:, inv]
        out[b] = (num / den).T
    return out, res


def kernel(x, Wq, Wk, Wv):
    out, _ = run(x, Wq, Wk, Wv, trace=False)
    return out
